# revision 63
# baseline (speedup 1.0000x reference)
"""Trainium2 Bass kernel for the OFPenalty eigenvalue-penalty loss.

Math (per sample b of 256):
  W = x[b] reshaped [C=2048, N=49];  G = W^T W  (49x49 Gram matrix)
  run1: x9 = G^9 x0 (power iteration, normalization deferred - scale
        invariant), largest = Rayleigh(G, x9) = x9^T (G x9) / x9^T x9
  run2: B = G - largest*I, u9 = B^9 x9, tmp = Rayleigh(B, u9)
  penalty = (tmp / (tmp + largest))^2 ; output = mean over batch.

Distribution: pure data parallel, 32 samples per core on 8 cores.
Samples are processed in pairs packed block-diagonally: sample 2p on
partitions 0:49, sample 2p+1 on partitions 64:113.

Performance structure (per core):
  - x is staged to fp16 on the host (partition-major), halving the DMA
    floor to ~18us.  Ragged transfers (1+2*7+1 pairs, 3136B-per-partition
    descriptors) start the gram stream early and shorten its tail.
  - Gram matmuls run in fp16 (1 PE cycle/row instead of fp32's 4); the
    gram phase + per-pair A^2 squaring builds (fp32, deferred two pairs
    so their operands are always ready) stream at DMA pace with the
    in-order PE queue free of dependency stalls.  ~3.5us of dummy
    matmuls at t=0 ramp the PE to peak clock before the first gram.
  - The power-iteration chains stay in fp32 (the per-sample penalty is
    ill-conditioned; low-precision trajectories diverge) and run as ONE
    16-pair lockstep chain of 15 dependency levels after the grams:
      run1:  x9 = A2 A2 A2 A2 (A x0)  (5 wave levels, 2^-12 per copy)
      run2:  u9 via 4 distributed B^2 double-steps: A2 u and
             A(-2 lam 2^-12 u) accumulate in PSUM and the lam^2 2^-12 u
             term is fused into the PSUM->SBUF copy (tensor add), so
             each level costs one copy and no extra matmul;
      Rayleigh-1 block sums come pre-broadcast to all partitions via a
      full [128,128] mask matmul, so lambda needs no extra broadcast
      round trip; w2 = A u9 with lam folded into the final divide, and
      the divide + penalty run fused on the vector engine.
    Exact powers of two cancel in every Rayleigh quotient.
"""

import os
import sys
from contextlib import ExitStack

import numpy as np

for _p in ("/opt/trn_rl_repo",):
    if os.path.isdir(_p) and _p not in sys.path:
        sys.path.insert(0, _p)

import concourse.bass as bass  # noqa: E402  (import keeps bass registered)
import concourse.tile as tile  # noqa: E402
from concourse import bacc, mybir  # noqa: E402
from concourse.bass_utils import run_bass_kernel_spmd  # noqa: E402

F32 = mybir.dt.float32
F16 = mybir.dt.float16
ALU = mybir.AluOpType

B, C, N = 256, 2048, 49
NCORES = 8
BS = B // NCORES  # 32 samples per core
NPAIR = BS // 2  # 16 pairs
KT = C // 128  # 16 contraction tiles
PG = 128
B1 = 64  # partition base of the second sample in a pair
S12 = float(2.0**-12)  # per-copy rescale (exact; cancels in Rayleigh)
S30 = float(2.0**-30)  # Rayleigh-2 denominator rescale (cancels exactly)
S60 = float(2.0**-60)  # Rayleigh-2 numerator rescale (= S30^2; cancels)
W = NPAIR  # chain width: all pairs in lockstep


def _emit(tc, x16, x0c, pen):
    nc = tc.nc
    ctx = ExitStack()
    with ctx:
        const = ctx.enter_context(tc.tile_pool(name="const", bufs=1))
        xpool = ctx.enter_context(tc.tile_pool(name="xt", bufs=5))
        vpool = ctx.enter_context(tc.tile_pool(name="vec", bufs=3))
        ps_ata = ctx.enter_context(tc.tile_pool(name="ps_ata", bufs=4, space="PSUM"))
        ps_bld = ctx.enter_context(tc.tile_pool(name="ps_bld", bufs=2, space="PSUM"))
        ps_wv = ctx.enter_context(tc.tile_pool(name="ps_wv", bufs=2, space="PSUM"))

        # ---- constants -------------------------------------------------
        X0 = const.tile([PG, NPAIR], F32)

        # PE p-state warmup: ~3.5us of dummy matmuls before the first gram
        # arrives, so the tensor engine is at peak clock from pair 0 on.
        JNK = const.tile([PG, PG], F16)
        nc.gpsimd.memset(JNK[:], 0.0)
        for i in range(28):
            psj = ps_bld.tile([PG, PG], F32, tag="bld", name=f"warm{i}")
            nc.tensor.matmul(psj[:], JNK[:], JNK[:], start=True, stop=True)

        CM01 = const.tile([PG, 2], F32)
        nc.gpsimd.memset(CM01[:], 0.0)
        nc.gpsimd.memset(CM01[0:N, 0:1], 1.0)
        nc.gpsimd.memset(CM01[B1 : B1 + N, 1:2], 1.0)

        CMF = const.tile([PG, PG], F32)
        nc.gpsimd.memset(CMF[:], 0.0)
        nc.gpsimd.memset(CMF[0:N, 0:B1], 1.0)
        nc.gpsimd.memset(CMF[B1 : B1 + N, B1:PG], 1.0)

        At = [const.tile([PG, PG], F32, tag=f"A{p}", name=f"A{p}")
              for p in range(NPAIR)]
        for p in range(NPAIR):
            nc.gpsimd.memset(At[p][:], 0.0)
        A2t = [const.tile([PG, PG], F32, tag=f"A2_{p}", name=f"A2_{p}")
               for p in range(NPAIR)]
        PENT = const.tile([2, NPAIR], F32)

        # ---- gram phase + pipelined A2 builds --------------------------
        # Two pairs per DMA (6272B per partition).  Per pair: fp16 gram
        # matmuls into two PSUM accumulators (PE column groups 0 and 64),
        # fp32 copies into the block-diagonal A tile, then A2 = (A@A)*2^-12
        # two pairs later so every matmul's operands are long ready before
        # the in-order PE reaches it.
        def emit_sq(p):
            ps2 = ps_bld.tile([PG, PG], F32, tag="bld", name=f"a2m{p}")
            nc.tensor.matmul(ps2[:], At[p][:], At[p][:], start=True, stop=True)
            if p % 2 == 0:
                nc.vector.tensor_scalar(A2t[p][:], ps2[:], S12, None,
                                        op0=ALU.mult)
            else:
                nc.scalar.mul(A2t[p][:], ps2[:], S12)

        # ragged DMA sizes: a 1-pair first transfer starts the gram stream
        # ~1.1us earlier, a 1-pair last transfer shortens the final burst.
        DMA_PAIRS = [1, 2, 2, 2, 2, 2, 2, 2, 1]
        p0 = 0
        for d, np_ in enumerate(DMA_PAIRS):
            xt = xpool.tile(
                [PG, np_ * 2 * KT * N], F16, tag=f"xt{np_}", name=f"xt{d}"
            )
            nc.sync.dma_start(xt[:], x16[:, p0 : p0 + np_, :])
            if d == 0:
                nc.scalar.dma_start(X0[:], x0c)
            for h in range(np_):
                p = p0 + h
                psa = ps_ata.tile([PG, N], F32, tag="ata", name=f"ata{p}a")
                psb = ps_ata.tile([PG, N], F32, tag="ata", name=f"ata{p}b")
                for k in range(KT):
                    for s in range(2):
                        pst = psa if s == 0 else psb
                        ob = 0 if s == 0 else B1
                        off = (2 * h + s) * (KT * N)
                        wk = xt[:, off + k * N : off + (k + 1) * N]
                        nc.tensor.matmul(
                            pst[ob : ob + N, :], wk, wk,
                            start=(k == 0), stop=(k == KT - 1),
                        )
                A = At[p][:]
                nc.vector.tensor_copy(A[0:N, 0:N], psa[0:N, :])
                nc.scalar.copy(A[B1 : B1 + N, B1 : B1 + N], psb[B1 : B1 + N, :])
                if p >= 2:
                    emit_sq(p - 2)
            p0 += np_
        for p in range(NPAIR - 2, NPAIR):
            emit_sq(p)

        Av = [At[p][:] for p in range(NPAIR)]
        A2v = [A2t[p][:] for p in range(NPAIR)]

        # ---- chain helpers ---------------------------------------------
        def chain_ps(name):
            return ps_wv.tile([PG, 2 * W], F32, tag="wv", name=name)

        def vtile(tag, name=None):
            return vpool.tile([PG, W], F32, tag=tag, name=name or tag)

        def matvecs(views, cur, tag):
            psw = chain_ps(f"wv_{tag}")
            for j in range(W):
                nc.tensor.matmul(psw[:, j : j + 1], views[j], cur[:, j : j + 1],
                                 start=True, stop=True)
            return psw

        st = {}

        # ---- run 1: x9 = A4 A4 (A x0), *2^-12 per wave -----------------
        def wave(views, src, dst, eng, tag):
            psw = matvecs(views, st[src], tag)
            nxt = vtile("v", f"v_{dst}")
            if eng == 0:
                nc.vector.tensor_scalar(nxt[:], psw[:, 0:W], S12, None,
                                        op0=ALU.mult)
            else:
                nc.scalar.mul(nxt[:], psw[:, 0:W], S12)
            st[dst] = nxt[:]

        st["x0"] = X0[:]
        wave(Av, "x0", "v1", 0, "v1")
        wave(A2v, "v1", "v3", 0, "v3")
        wave(A2v, "v3", "v5", 0, "v5")
        wave(A2v, "v5", "v7", 0, "v7")
        wave(A2v, "v7", "x9", 0, "x9")

        # ---- Rayleigh 1 -> LAM -----------------------------------------
        TT1 = vpool.tile([PG, 2 * W], F32, tag="tt", name="tt_r1")
        nc.scalar.square(TT1[:, W : 2 * W], st["x9"])
        psww = matvecs(Av, st["x9"], "w")
        nc.vector.tensor_mul(TT1[:, 0:W], st["x9"], psww[:, 0:W])

        pdv = chain_ps("pdv_r1")
        nc.tensor.matmul(pdv[:], CMF[:], TT1[:], start=True, stop=True)
        pd1 = chain_ps("pd_r1")
        nc.tensor.matmul(pd1[0:2, :], CM01[:], TT1[:], start=True, stop=True)

        # per-partition lambda directly from the broadcast sums
        RV = vtile("rv", "rv")
        nc.vector.reciprocal(RV[:], pdv[:, W : 2 * W])
        LV = vtile("lv", "lv")
        nc.vector.tensor_mul(LV[:], pdv[:, 0:W], RV[:])
        hx = vtile("t1", "hx")
        nc.vector.tensor_mul(hx[:], LV[:], st["x9"])

        # ---- run 2: u1 = A x9 - lam x9, then 4 distributed B^2 steps ---
        # u_{k+2} = (A2 u) + A(-2 lam S12 u) + I(lam^2 S12 u)
        #         = B^2 u * 2^-12   (A2 carries one 2^-12 already);
        # each level: 2 DVE pre-products, 3 accumulated matvecs per pair,
        # one PSUM->SBUF copy.
        psu1 = chain_ps("wv_u1")
        for j in range(W):
            nc.tensor.matmul(psu1[:, j : j + 1], Av[j],
                             st["x9"][:, j : j + 1], start=True, stop=True)
        u1 = vtile("v", "u1")
        nc.vector.tensor_sub(u1[:], psu1[:, 0:W], hx[:])
        st["u1"] = u1[:]
        # off the critical path: scalar LAM for the final divide, and the
        # per-step shift coefficients
        ND1 = vpool.tile([2, 2 * W], F32, tag="nd", name="nd_r1")
        RD1 = vpool.tile([2, W], F32, tag="rd", name="rd_r1")
        LAM = vpool.tile([2, W], F32, tag="lam", name="lam")
        nc.vector.tensor_copy(ND1[:], pd1[0:2, :])
        nc.vector.reciprocal(RD1[:], ND1[:, W : 2 * W])
        nc.vector.tensor_mul(LAM[:], ND1[:, 0:W], RD1[:])
        NLV2S = vtile("lv2", "nlv2s")
        nc.vector.tensor_scalar(NLV2S[:], LV[:], -2.0 * S12, None, op0=ALU.mult)
        SQl = vtile("sq", "sql")
        nc.vector.tensor_mul(SQl[:], LV[:], LV[:])
        LVQS = vtile("lvq", "lvqs")
        nc.vector.tensor_scalar(LVQS[:], SQl[:], S12, None, op0=ALU.mult)

        for i, (src, dst) in enumerate(
            [("u1", "u3"), ("u3", "u5"), ("u5", "u7"), ("u7", "u9")]
        ):
            hv = vtile("e", f"h_{dst}")
            nc.vector.tensor_mul(hv[:], NLV2S[:], st[src])
            gv = vtile("t1", f"g_{dst}")
            nc.vector.tensor_mul(gv[:], LVQS[:], st[src])
            psw = chain_ps(f"wv_{dst}")
            for j in range(W):
                nc.tensor.matmul(psw[:, j : j + 1], A2v[j],
                                 st[src][:, j : j + 1], start=True, stop=False)
                nc.tensor.matmul(psw[:, j : j + 1], Av[j],
                                 hv[:, j : j + 1], start=False, stop=True)
            nxt = vtile("v", dst)
            nc.vector.tensor_add(nxt[:], psw[:, 0:W], gv[:])
            st[dst] = nxt[:]

        # ---- Rayleigh 2 (w2 = A u9; lam folded into the divide) --------
        # num = (u9*S60) . (A u9), den = (u9*S30)^2: scales cancel in the
        # quotient and the numerator needs only one DVE op off the PSUM.
        US = vtile("us", "us")
        nc.scalar.mul(US[:], st["u9"], S30)
        US6 = vtile("us6", "us6")
        nc.vector.tensor_scalar(US6[:], st["u9"], S60, None, op0=ALU.mult)
        psw2 = matvecs(Av, st["u9"], "w2")
        TT2 = vpool.tile([PG, 2 * W], F32, tag="tt", name="tt_r2")
        nc.scalar.square(TT2[:, W : 2 * W], US[:])
        nc.vector.tensor_mul(TT2[:, 0:W], US6[:], psw2[:, 0:W])

        pd2 = chain_ps("pd_r2")
        nc.tensor.matmul(pd2[0:2, :], CM01[:], TT2[:], start=True, stop=True)

        # tmp = u9.A.u9/u9.u9 - lam ; sm = tmp + lam; pen = (tmp/sm)^2
        RD2 = vpool.tile([2, W], F32, tag="rd", name="rd_r2")
        T0 = vpool.tile([2, W], F32, tag="t0", name="t0")
        DF = vpool.tile([2, W], F32, tag="df", name="df")
        RS = vpool.tile([2, W], F32, tag="rs", name="rs")
        RT = vpool.tile([2, W], F32, tag="rt", name="rt")
        nc.vector.reciprocal(RD2[:], pd2[0:2, W : 2 * W])
        nc.vector.tensor_mul(T0[:], pd2[0:2, 0:W], RD2[:])
        nc.vector.tensor_sub(DF[:], T0[:], LAM[:])
        nc.vector.reciprocal(RS[:], T0[:])
        nc.vector.tensor_mul(RT[:], DF[:], RS[:])
        nc.vector.tensor_mul(PENT[:], RT[:], RT[:])

        nc.sync.dma_start(pen, PENT[:])


_NC_CACHE = {}


def build_nc():
    if "nc" in _NC_CACHE:
        return _NC_CACHE["nc"]
    nc = bacc.Bacc("TRN2", target_bir_lowering=False, debug=False)
    x16 = nc.dram_tensor("x16", [PG, NPAIR, 2 * KT * N], F16, kind="ExternalInput")
    x0c = nc.dram_tensor("x0c", [PG, NPAIR], F32, kind="ExternalInput")
    pen = nc.dram_tensor("pen", [2, NPAIR], F32, kind="ExternalOutput")
    with tile.TileContext(nc) as tc:
        _emit(tc, x16.ap(), x0c.ap(), pen.ap())
    nc.compile()
    _NC_CACHE["nc"] = nc
    return nc


LAST_RESULTS = None


def kernel(x, x0):
    global LAST_RESULTS
    x = np.asarray(x, dtype=np.float32).reshape(B, C, N)
    x0 = np.asarray(x0, dtype=np.float32).reshape(B, N)

    # host staging: fp16 pair tiles in gram k-tile layout.
    # channel c = 512*b + 4*q + r -> partition q holds, per sample, the
    # 784 values [b, r, j] contiguously (3136B descriptors).
    xr = x.reshape(NCORES, NPAIR, 2, 4, PG, 4, N)
    x16 = np.ascontiguousarray(
        xr.transpose(0, 4, 1, 2, 3, 5, 6).reshape(
            NCORES, PG, NPAIR, 2 * KT * N
        )
    ).astype(np.float16)

    x0r = x0.reshape(NCORES, NPAIR, 2, N)
    x0c = np.zeros((NCORES, PG, NPAIR), dtype=np.float32)
    x0c[:, 0:N, :] = x0r[:, :, 0, :].transpose(0, 2, 1)
    x0c[:, B1 : B1 + N, :] = x0r[:, :, 1, :].transpose(0, 2, 1)

    nc = build_nc()
    in_maps = [{"x16": x16[i], "x0c": x0c[i]} for i in range(NCORES)]
    trace = bool(int(os.environ.get("KERNEL_TRACE", "0")))
    res = run_bass_kernel_spmd(nc, in_maps, list(range(NCORES)), trace=trace)
    LAST_RESULTS = res
    # pen[s, p] = penalty of sample 2p+s on that core
    pens = np.concatenate(
        [r["pen"].reshape(2, NPAIR).T.reshape(-1) for r in res.results]
    )
    return np.float32(pens.sum(dtype=np.float64) / B)
